# revision 22
# baseline (speedup 1.0000x reference)
"""Multi-head attention TRN2 kernel (8 NeuronCores).

Sharding: core (2b + h2) handles batch b (of 4) and head-half h2 (8 of 16
heads).  Each core projects its batch's Q/K/V through its 512-column slice
of Wq/Wk/Wv, runs causal flash-attention for its 8 heads, and computes a
partial output projection through its 512 rows of Wo^T.  The two partial
outputs per batch are summed on the host (the "all-reduce after W_o"),
plus the output bias.

All matmuls run in fp16 with fp32 PSUM accumulation.  Scores are computed
transposed (S^T[kj, qi] = kT.T @ qT) so the softmax sum comes for free from
a ones-column appended to V (padded to 128 columns so LDWEIGHTS gets fast
weight load), and the attention output lands f-major, which is exactly the
lhsT layout the Wo matmul needs.  Normalisation (divide by the softmax sum)
is a DVE fast-reciprocal + GPSIMD partition-broadcast + DVE multiply.
Causal masking multiplies the 4 diagonal tiles by precomputed 0/1 strips on
the DVE; off-diagonal upper tiles are skipped entirely.

The per-chunk pipeline (project K/Q/V for a 512-token chunk -> attention
for all 8 local heads on that qi block -> partial Wo) interleaves dense PE
segments with the ACT-paced attention inner loop so the PE HAM clock stays
at 2.4 GHz; exp runs on 1024-wide [128,2,512] PSUM tiles to amortise the
ACT per-instruction overhead.
"""

import os
import sys
import time

sys.path.insert(0, "/opt/trn_rl_repo")

import numpy as np

import concourse.bass as bass
import concourse.mybir as mybir
import concourse.tile as tile
from concourse import bacc
from concourse.bass_utils import run_bass_kernel_spmd
from concourse.masks import make_identity

F16 = mybir.dt.float16
F32 = mybir.dt.float32
P = 128

# Problem constants (full size).
D_MODEL = 1024
NUM_HEADS = 16
DK = D_MODEL // NUM_HEADS  # 64
BATCH = 4
SEQ = 2048
N_CORES = 8

LAST_EXEC_NS = None
LAST_RESULTS = None


def build_program(seq=SEQ, d_model=D_MODEL, num_heads=NUM_HEADS, mode="causal"):
    """Build the per-core Bass program.  Uniform across cores (SPMD).

    mode: "causal" (tril mask, block-skip + affine_select on diagonal),
          "dense"  (no mask),
          "mask"   (arbitrary 0/1 mask, multiplicative, streamed from DRAM).
    """
    assert d_model % 256 == 0 and seq % P == 0
    HL = num_heads // 2              # local heads per core
    PAIRS = HL // 2                  # head-pairs (128 partitions each)
    FL = HL * DK                     # local features (columns of W slices)
    IN_T = d_model // P              # input-dim tiles
    FT = FL // P                     # local f tiles
    TT = seq // P                    # token tiles
    QBS = min(512, seq)              # qi block size
    QB = seq // QBS                  # qi blocks
    KJ = seq // P                    # key tiles
    KPB = QBS // P                   # key tiles per qi block (diag width)
    OFC = (d_model + 511) // 512     # output-feature chunks
    OFS = min(512, d_model)
    assert PAIRS >= 1 and FT >= 1 and QB >= 1

    nc = bacc.Bacc()
    xtq = nc.declare_dram_parameter("xtq", [d_model, seq], F16, isOutput=False)
    xtk = nc.declare_dram_parameter("xtk", [d_model, seq], F16, isOutput=False)
    xtv = nc.declare_dram_parameter("xtv", [d_model, seq], F16, isOutput=False)
    wqt = nc.declare_dram_parameter("wqt", [d_model, FL], F16, isOutput=False)
    wkt = nc.declare_dram_parameter("wkt", [d_model, FL], F16, isOutput=False)
    wvt = nc.declare_dram_parameter("wvt", [d_model, FL], F16, isOutput=False)
    bqr = nc.declare_dram_parameter("bqr", [P, PAIRS], F32, isOutput=False)
    bkr = nc.declare_dram_parameter("bkr", [P, PAIRS], F32, isOutput=False)
    bvrow = nc.declare_dram_parameter("bvrow", [1, FL], F16, isOutput=False)
    wot = nc.declare_dram_parameter("wot", [FL, d_model], F16, isOutput=False)
    if mode == "mask":
        maskt = nc.declare_dram_parameter("maskt", [seq, seq], F16, isOutput=False)
    out = nc.declare_dram_parameter("out", [seq, d_model], F32, isOutput=True)

    AF = mybir.ActivationFunctionType

    with tile.TileContext(nc) as tc:
        with (
            tc.tile_pool(name="const", bufs=1) as cpool,
            tc.tile_pool(name="big", bufs=1) as big,
            tc.tile_pool(name="xs", bufs=2) as xs,
            tc.tile_pool(name="es", bufs=6) as esp,
            tc.tile_pool(name="ep", bufs=2) as epi,
            tc.tile_pool(name="osb", bufs=3) as osb,
        ):
            # ---- constants ----
            wk_sb = cpool.tile([P, IN_T, FL], F16)
            nc.sync.dma_start(wk_sb[:], wkt.rearrange("(it p) f -> p it f", p=P))
            wq_sb = cpool.tile([P, IN_T, FL], F16)
            nc.sync.dma_start(wq_sb[:], wqt.rearrange("(it p) f -> p it f", p=P))
            # prefetch chunk-0 activations before the remaining weights so
            # the first K-projection can start as early as possible
            xk0 = xs.tile([P, IN_T, QBS], F16, tag="xk", name="xk_0")
            nc.sync.dma_start(
                xk0[:], xtk[:, 0:QBS].rearrange("(it p) t -> p it t", p=P))
            xq0 = xs.tile([P, IN_T, QBS], F16, tag="xq", name="xq_0")
            nc.sync.dma_start(
                xq0[:], xtq[:, 0:QBS].rearrange("(it p) t -> p it t", p=P))
            xv0 = xs.tile([P, IN_T, QBS], F16, tag="xv", name="xv_0")
            nc.sync.dma_start(
                xv0[:], xtv[:, 0:QBS].rearrange("(it p) t -> p it t", p=P))
            wv_sb = cpool.tile([P, IN_T, FL], F16)
            nc.sync.dma_start(wv_sb[:], wvt.rearrange("(it p) f -> p it f", p=P))
            wo_sb = cpool.tile([P, FT, d_model], F16)
            nc.sync.dma_start(wo_sb[:], wot.rearrange("(ft p) o -> p ft o", p=P))
            bqr_sb = cpool.tile([P, PAIRS], F32)
            nc.sync.dma_start(bqr_sb[:], bqr[:, :])
            bkr_sb = cpool.tile([P, PAIRS], F32)
            nc.sync.dma_start(bkr_sb[:], bkr[:, :])
            bv_sb = cpool.tile([1, FL], F16)
            nc.sync.dma_start(bv_sb[:], bvrow[:, :])
            ones1 = cpool.tile([1, P], F16)
            nc.gpsimd.memset(ones1[:], 1.0)
            # 4 diagonal 0/1 strips: strip j keeps (qi - kj_local - 128*j >= 0)
            mask4 = cpool.tile([P, KPB, QBS], F16)
            nc.gpsimd.memset(mask4[:], 1.0)
            for j in range(KPB):
                nc.gpsimd.affine_select(
                    out=mask4[:, j, :], in_=mask4[:, j, :],
                    compare_op=mybir.AluOpType.is_ge,
                    fill=0.0, base=-P * j,
                    pattern=[[1, QBS]], channel_multiplier=-1)

            # ---- persistent activations ----
            qT_sb = big.tile([P, PAIRS, seq], F16)   # [2-head f, pair, tok]
            kT_sb = big.tile([P, PAIRS, seq], F16)
            v_sb = big.tile([P, TT, HL, P], F16)  # [tok_in_tile, kj, h, d|1|pad]
            oT_sb = big.tile([P, FT, seq], F16)      # attention out, f-major

            nc.gpsimd.memset(v_sb[:], 0.0)
            nc.gpsimd.memset(v_sb[:, :, :, DK:DK + 1], 1.0)

            # warm the ACT exp table early (one-time ~2.7us load)
            es_warm = esp.tile([1, 8], F16, tag="warm")
            nc.scalar.activation(es_warm[:], ones1[0:1, 0:8], AF.Exp, scale=1.0)

            # One shared PSUM pool: tag "s" [128,2,512]x3 = 6 banks (scores,
            # projections, Wo) + tag "o" [65,512]x2 = 2 banks.
            pool_cm = tc.tile_pool(name="pmm", bufs=3, space="PSUM")
            pmm = pool_cm.__enter__()
            opool_cm = tc.tile_pool(name="po", bufs=2, space="PSUM")
            pop = opool_cm.__enter__()

            # qb-outer software pipeline: per 512-token chunk, project
            # K/Q/V for that chunk, run attention for all local heads on
            # that qi block, then the (partial) output projection for it.
            # Dense projection segments interleave with the ACT-paced
            # attention segments, keeping the PE HAM-warm throughout; the
            # next chunk's projections are emitted right after a chunk's
            # attention so they can fill attention-tail PE stalls.
            def emit_proj_dma(ch):
                    tsl = slice(ch * QBS, (ch + 1) * QBS)
                    xk_t = xs.tile([P, IN_T, QBS], F16, tag="xk",
                                   name=f"xk_{ch}")
                    nc.sync.dma_start(
                        xk_t[:], xtk[:, tsl].rearrange("(it p) t -> p it t", p=P))
                    xq_t = xs.tile([P, IN_T, QBS], F16, tag="xq",
                                   name=f"xq_{ch}")
                    nc.sync.dma_start(
                        xq_t[:], xtq[:, tsl].rearrange("(it p) t -> p it t", p=P))
                    xv_t = xs.tile([P, IN_T, QBS], F16, tag="xv",
                                   name=f"xv_{ch}")
                    nc.sync.dma_start(
                        xv_t[:], xtv[:, tsl].rearrange("(it p) t -> p it t", p=P))
                    return xk_t, xq_t, xv_t

            def proj_pieces(ch, tiles=None):
                    tsl = slice(ch * QBS, (ch + 1) * QBS)
                    xk_t, xq_t, xv_t = tiles if tiles else emit_proj_dma(ch)
                    pieces = []
                    def qk_piece(pair):
                        def go():
                            _emit_qk_pair(ch, tsl, xk_t, xq_t, pair)
                        return go
                    def v_piece(tl):
                        def go():
                            _emit_v_pair(ch, tsl, xv_t, tl)
                        return go
                    for pair in range(PAIRS):
                        pieces.append(qk_piece(pair))
                    for tl in range(0, KPB, 2):
                        pieces.append(v_piece(tl))
                    return pieces

            def emit_proj(ch, tiles=None):
                    for piece in proj_pieces(ch, tiles):
                        piece()

            def _emit_qk_pair(ch, tsl, xk_t, xq_t, pair):
                        fsl = slice(pair * P, (pair + 1) * P)
                        qk_ps = pmm.tile([P, 2, QBS], F32, tag="s",
                                         name=f"qk_{ch}_{pair}")
                        for it in range(IN_T):
                            nc.tensor.matmul(qk_ps[:, 0, :], wk_sb[:, it, fsl],
                                             xk_t[:, it, :],
                                             start=(it == 0), stop=(it == IN_T - 1))
                        for it in range(IN_T):
                            nc.tensor.matmul(qk_ps[:, 1, :], wq_sb[:, it, fsl],
                                             xq_t[:, it, :],
                                             start=(it == 0), stop=(it == IN_T - 1))
                        nc.vector.tensor_scalar_add(kT_sb[:, pair, tsl],
                                                    qk_ps[:, 0, :],
                                                    bkr_sb[:, pair:pair + 1])
                        nc.vector.tensor_scalar_add(qT_sb[:, pair, tsl],
                                                    qk_ps[:, 1, :],
                                                    bqr_sb[:, pair:pair + 1])

            def _emit_v_pair(ch, tsl, xv_t, tl):
                        v_ps = pmm.tile([P, 2, QBS], F32, tag="s",
                                        name=f"v_{ch}_{tl}")
                        for i in range(2):
                            tt = ch * KPB + tl + i
                            nc.tensor.matmul(v_ps[:, i, 0:FL], ones1[0:1, :],
                                             bv_sb[0:1, :], start=True, stop=False)
                            for it in range(IN_T):
                                nc.tensor.matmul(
                                    v_ps[:, i, 0:FL],
                                    xv_t[:, it, (tl + i) * P:(tl + i + 1) * P],
                                    wv_sb[:, it, :],
                                    start=False, stop=(it == IN_T - 1))
                        for i in range(2):
                            tt = ch * KPB + tl + i
                            nc.vector.tensor_copy(
                                v_sb[:, tt, :, 0:DK],
                                v_ps[:, i, 0:FL].rearrange("p (h d) -> p h d", h=HL))

            # ---- attention for qi block qb, all local heads ----
            def emit_attn(qb, fillers=()):
                fillers = list(fillers)
                qsl = slice(qb * QBS, (qb + 1) * QBS)
                KL = (qb + 1) * KPB if mode == "causal" else KJ
                for h in range(HL):
                    pair = h // 2
                    po = (h % 2) * DK
                    qT_h = qT_sb[po:po + DK, pair, qsl]
                    o_ps = pop.tile([P, QBS], F32, tag="o",
                                    name=f"o_{qb}_{h}")
                    for kj0 in range(0, KL, 2):
                        kjs = [kj0, kj0 + 1] if kj0 + 1 < KL else [kj0]
                        s_ps = pmm.tile([P, 2, QBS], F32, tag="s",
                                        name=f"s_{qb}_{h}_{kj0}")
                        for i, kj in enumerate(kjs):
                            nc.tensor.matmul(
                                s_ps[:, i, :],
                                kT_sb[po:po + DK, pair, kj * P:(kj + 1) * P],
                                qT_h, start=True, stop=True)
                        es = esp.tile([P, 2, QBS], F16, tag="es",
                                      name=f"es_{qb}_{h}_{kj0}")
                        n = len(kjs)
                        nc.scalar.activation(es[:, :n, :], s_ps[:, :n, :],
                                             AF.Exp, scale=0.125)
                        for i, kj in enumerate(kjs):
                            if mode == "causal" and kj // KPB == qb:
                                nc.vector.tensor_mul(es[:, i, :], es[:, i, :],
                                                     mask4[:, kj % KPB, :])
                            elif mode == "mask":
                                m_t = esp.tile([P, QBS], F16, tag="mt")
                                nc.sync.dma_start(
                                    m_t[:], maskt[kj * P:(kj + 1) * P, qsl])
                                nc.vector.tensor_mul(es[:, i, :], es[:, i, :],
                                                     m_t[:])
                        for i, kj in enumerate(kjs):
                            nc.tensor.matmul(o_ps[:], v_sb[:, kj, h, :],
                                             es[:, i, :],
                                             start=(kj == 0), stop=(kj == KL - 1))
                    # normalise by the softmax sum (row DK), f-major
                    srow = epi.tile([1, QBS], F32, tag="srow")
                    nc.vector.tensor_copy(srow[:], o_ps[DK:DK + 1, :])
                    recip_row = epi.tile([1, QBS], F32, tag="recip_row")
                    nc.vector.reciprocal_approx_fast(recip_row[:], srow[:])
                    recipb = epi.tile([DK, QBS], F32, tag="recipb")
                    nc.gpsimd.partition_broadcast(recipb[:], recip_row[0:1, :])
                    nc.vector.tensor_mul(oT_sb[po:po + DK, pair, qsl],
                                         o_ps[0:DK, :], recipb[:])
                    if fillers:
                        fillers.pop(0)()
                for f in fillers:
                    f()

            # ---- output projection for one token chunk ----
            def wo_pieces(qb):
                def tt_piece(tl):
                    def go():
                        _emit_wo_tt(qb, tl)
                    return go
                return [tt_piece(tl) for tl in range(KPB)]

            def emit_wo(qb):
                for piece in wo_pieces(qb):
                    piece()

            def _emit_wo_tt(qb, tl):
                    tt = qb * KPB + tl
                    w_ps = pmm.tile([P, 2, QBS], F32, tag="s", name=f"w_{tt}")
                    for ofc in range(OFC):
                        osl = slice(ofc * OFS, (ofc + 1) * OFS)
                        for ft in range(FT):
                            nc.tensor.matmul(w_ps[:, ofc, 0:OFS],
                                             oT_sb[:, ft, tt * P:(tt + 1) * P],
                                             wo_sb[:, ft, osl],
                                             start=(ft == 0), stop=(ft == FT - 1))
                    o_out = osb.tile([P, OFC, OFS], F32, tag="oo")
                    nc.vector.tensor_copy(o_out[:], w_ps[:, 0:OFC, 0:OFS])
                    nc.sync.dma_start(
                        out[tt * P:(tt + 1) * P, :],
                        o_out[:].rearrange("p c o -> p (c o)"))

            if mode == "causal":
                emit_proj(0, (xk0, xq0, xv0))
                if QB > 1:
                    emit_proj(1)
                for qb in range(QB):
                    fillers = []
                    if qb + 2 < QB:
                        fillers += proj_pieces(qb + 2)
                    if qb >= 1:
                        fillers += wo_pieces(qb - 1)
                    emit_attn(qb, fillers)
                emit_wo(QB - 1)
            else:
                for ch in range(QB):
                    emit_proj(ch, (xk0, xq0, xv0) if ch == 0 else None)
                for qb in range(QB):
                    emit_attn(qb)
                    emit_wo(qb)

            opool_cm.__exit__(None, None, None)
            pool_cm.__exit__(None, None, None)

    nc.compile()
    return nc


_PROGRAMS = {}


def _get_program(mode, seq=SEQ, d_model=D_MODEL, num_heads=NUM_HEADS):
    key = (mode, seq, d_model, num_heads)
    if key not in _PROGRAMS:
        _PROGRAMS[key] = build_program(seq, d_model, num_heads, mode)
    return _PROGRAMS[key]


def _detect_mode(mask, seq):
    m = np.asarray(mask)
    if (m != 0).all():
        return "dense"
    tril = np.tril(np.ones((seq, seq), np.int8))
    if np.array_equal((m != 0).astype(np.int8), tril):
        return "causal"
    return "mask"


def prep_inputs(Q, K, V, mask, Wq, bq, Wk, bk, Wv, bv, Wo, bo,
                num_heads=NUM_HEADS, mode=None):
    batch, seq, d_model = Q.shape
    HL = num_heads // 2
    FL = HL * (d_model // num_heads)
    PAIRS = HL // 2
    if mode is None:
        mode = _detect_mode(mask, seq)
    maskt = None
    if mode == "mask":
        maskt = np.ascontiguousarray(
            (np.asarray(mask) != 0).astype(np.float16).T)
    in_maps = []
    for b in range(batch):
        xtq = Q[b].T.astype(np.float16)
        xtk = K[b].T.astype(np.float16)
        xtv = V[b].T.astype(np.float16)
        for half in range(2):
            fsl = slice(half * FL, (half + 1) * FL)
            im = {
                "xtq": xtq, "xtk": xtk, "xtv": xtv,
                "wqt": np.ascontiguousarray(Wq[fsl, :].T).astype(np.float16),
                "wkt": np.ascontiguousarray(Wk[fsl, :].T).astype(np.float16),
                "wvt": np.ascontiguousarray(Wv[fsl, :].T).astype(np.float16),
                "bqr": np.ascontiguousarray(
                    bq[fsl].reshape(PAIRS, P).T).astype(np.float32),
                "bkr": np.ascontiguousarray(
                    bk[fsl].reshape(PAIRS, P).T).astype(np.float32),
                "bvrow": bv[fsl].reshape(1, FL).astype(np.float16),
                "wot": np.ascontiguousarray(Wo[:, fsl].T).astype(np.float16),
            }
            if maskt is not None:
                im["maskt"] = maskt
            in_maps.append(im)
    return in_maps, mode


def _install_trace_hooks():
    """Provide antenv.axon_hooks (missing in this image) so that
    run_bass_kernel_spmd(trace=True) can capture NTFF profiles via the
    axon PJRT .so.  Bench-only; the graded path never enables tracing."""
    import contextlib
    import ctypes
    import types
    try:
        from antenv import axon_hooks  # noqa: F401
        return
    except ImportError:
        pass
    lib = ctypes.CDLL("/opt/axon/libaxon_pjrt.so")
    if not hasattr(lib, "axon_start_nrt_profile"):
        return
    lib.axon_start_nrt_profile.argtypes = [ctypes.POINTER(ctypes.c_int64),
                                           ctypes.c_size_t]
    lib.axon_start_nrt_profile.restype = ctypes.c_int64
    lib.axon_stop_nrt_profile.argtypes = [ctypes.c_char_p]
    lib.axon_stop_nrt_profile.restype = ctypes.c_int64

    @contextlib.contextmanager
    def _hook(output_dir, device_ids):
        import jax
        jax.devices()
        if device_ids:
            ids = (ctypes.c_int64 * len(device_ids))(*device_ids)
            rc = lib.axon_start_nrt_profile(ids, len(device_ids))
        else:
            rc = lib.axon_start_nrt_profile(None, 0)
        if rc != 0:
            raise RuntimeError(f"axon_start_nrt_profile rc={rc}")
        try:
            yield
        finally:
            n = lib.axon_stop_nrt_profile(str(output_dir).encode())
            print(f"profile: {n} file(s) written to {output_dir}", file=sys.stderr)

    mod = types.ModuleType("antenv.axon_hooks")
    mod.get_axon_ntff_profile_hook = lambda: _hook
    mod.set_axon_ntff_profile_hook = lambda h: None
    sys.modules["antenv.axon_hooks"] = mod
    import concourse.bass_utils as bu
    bu.upload_artifacts = lambda tmpdir: f"local:{tmpdir}"


def kernel(Q, K, V, mask, Wq, bq, Wk, bk, Wv, bv, Wo, bo):
    global LAST_EXEC_NS, LAST_RESULTS
    Q = np.asarray(Q); K = np.asarray(K); V = np.asarray(V)
    mask = np.asarray(mask)
    Wq = np.asarray(Wq, np.float32); bq = np.asarray(bq, np.float32)
    Wk = np.asarray(Wk, np.float32); bk = np.asarray(bk, np.float32)
    Wv = np.asarray(Wv, np.float32); bv = np.asarray(bv, np.float32)
    Wo = np.asarray(Wo, np.float32); bo = np.asarray(bo, np.float32)
    batch, seq, d_model = Q.shape

    in_maps, mode = prep_inputs(Q, K, V, mask, Wq, bq, Wk, bk, Wv, bv, Wo, bo)
    nc = _get_program(mode, seq, d_model, NUM_HEADS)

    trace = bool(os.environ.get("KBENCH_TRACE"))
    tmpdir = os.environ.get("KBENCH_TRACE_DIR") or None
    if trace:
        _install_trace_hooks()
    res = run_bass_kernel_spmd(nc, in_maps, list(range(N_CORES)), trace=trace,
                               tmpdir=tmpdir)
    LAST_EXEC_NS = res.exec_time_ns
    LAST_RESULTS = res
    out = np.empty((batch, seq, d_model), np.float32)
    for b in range(batch):
        out[b] = res.results[2 * b]["out"] + res.results[2 * b + 1]["out"] + bo
    return out


# revision 23
# speedup vs baseline: 1.0371x; 1.0371x over previous
"""Multi-head attention TRN2 kernel (8 NeuronCores).

Sharding: core (2b + h2) handles batch b (of 4) and head-half h2 (8 of 16
heads).  Each core projects its batch's Q/K/V through its 512-column slice
of Wq/Wk/Wv, runs causal flash-attention for its 8 heads, and computes a
partial output projection through its 512 rows of Wo^T.  The two partial
outputs per batch are summed on the host (the "all-reduce after W_o"),
plus the output bias.

All matmuls run in fp16 with fp32 PSUM accumulation.  Scores are computed
transposed (S^T[kj, qi] = kT.T @ qT) so the softmax sum comes for free from
a ones-column appended to V (padded to 128 columns so LDWEIGHTS gets fast
weight load), and the attention output lands f-major, which is exactly the
lhsT layout the Wo matmul needs.  Normalisation (divide by the softmax sum)
is a DVE fast-reciprocal + GPSIMD partition-broadcast + DVE multiply.
Causal masking multiplies the 4 diagonal tiles by precomputed 0/1 strips on
the DVE; off-diagonal upper tiles are skipped entirely.

The per-chunk pipeline (project K/Q/V for a 512-token chunk -> attention
for all 8 local heads on that qi block -> partial Wo) interleaves dense PE
segments with the ACT-paced attention inner loop so the PE HAM clock stays
at 2.4 GHz; exp runs on 1024-wide [128,2,512] PSUM tiles to amortise the
ACT per-instruction overhead.
"""

import os
import sys
import time

sys.path.insert(0, "/opt/trn_rl_repo")

import numpy as np

import concourse.bass as bass
import concourse.mybir as mybir
import concourse.tile as tile
from concourse import bacc
from concourse.bass_utils import run_bass_kernel_spmd
from concourse.masks import make_identity

F16 = mybir.dt.float16
F32 = mybir.dt.float32
P = 128

# Problem constants (full size).
D_MODEL = 1024
NUM_HEADS = 16
DK = D_MODEL // NUM_HEADS  # 64
BATCH = 4
SEQ = 2048
N_CORES = 8

LAST_EXEC_NS = None
LAST_RESULTS = None


def build_program(seq=SEQ, d_model=D_MODEL, num_heads=NUM_HEADS, mode="causal"):
    """Build the per-core Bass program.  Uniform across cores (SPMD).

    mode: "causal" (tril mask, block-skip + affine_select on diagonal),
          "dense"  (no mask),
          "mask"   (arbitrary 0/1 mask, multiplicative, streamed from DRAM).
    """
    assert d_model % 256 == 0 and seq % P == 0
    HL = num_heads // 2              # local heads per core
    PAIRS = HL // 2                  # head-pairs (128 partitions each)
    FL = HL * DK                     # local features (columns of W slices)
    IN_T = d_model // P              # input-dim tiles
    FT = FL // P                     # local f tiles
    TT = seq // P                    # token tiles
    QBS = min(512, seq)              # qi block size
    QB = seq // QBS                  # qi blocks
    KJ = seq // P                    # key tiles
    KPB = QBS // P                   # key tiles per qi block (diag width)
    OFC = (d_model + 511) // 512     # output-feature chunks
    OFS = min(512, d_model)
    assert PAIRS >= 1 and FT >= 1 and QB >= 1

    nc = bacc.Bacc()
    xtq = nc.declare_dram_parameter("xtq", [d_model, seq], F16, isOutput=False)
    xtk = nc.declare_dram_parameter("xtk", [d_model, seq], F16, isOutput=False)
    xtv = nc.declare_dram_parameter("xtv", [d_model, seq], F16, isOutput=False)
    wqt = nc.declare_dram_parameter("wqt", [d_model, FL], F16, isOutput=False)
    wkt = nc.declare_dram_parameter("wkt", [d_model, FL], F16, isOutput=False)
    wvt = nc.declare_dram_parameter("wvt", [d_model, FL], F16, isOutput=False)
    bqr = nc.declare_dram_parameter("bqr", [P, PAIRS], F32, isOutput=False)
    bkr = nc.declare_dram_parameter("bkr", [P, PAIRS], F32, isOutput=False)
    bvrow = nc.declare_dram_parameter("bvrow", [1, FL], F16, isOutput=False)
    wot = nc.declare_dram_parameter("wot", [FL, d_model], F16, isOutput=False)
    if mode == "mask":
        maskt = nc.declare_dram_parameter("maskt", [seq, seq], F16, isOutput=False)
    out = nc.declare_dram_parameter("out", [seq, d_model], F32, isOutput=True)

    AF = mybir.ActivationFunctionType

    with tile.TileContext(nc) as tc:
        with (
            tc.tile_pool(name="const", bufs=1) as cpool,
            tc.tile_pool(name="big", bufs=1) as big,
            tc.tile_pool(name="xs", bufs=2) as xs,
            tc.tile_pool(name="es", bufs=6) as esp,
            tc.tile_pool(name="ep", bufs=2) as epi,
            tc.tile_pool(name="osb", bufs=3) as osb,
        ):
            # ---- constants ----
            wk_sb = cpool.tile([P, IN_T, FL], F16)
            nc.sync.dma_start(wk_sb[:], wkt.rearrange("(it p) f -> p it f", p=P))
            wq_sb = cpool.tile([P, IN_T, FL], F16)
            nc.sync.dma_start(wq_sb[:], wqt.rearrange("(it p) f -> p it f", p=P))
            # prefetch chunk-0 activations before the remaining weights so
            # the first K-projection can start as early as possible
            xk0 = xs.tile([P, IN_T, QBS], F16, tag="xk", name="xk_0")
            nc.sync.dma_start(
                xk0[:], xtk[:, 0:QBS].rearrange("(it p) t -> p it t", p=P))
            xq0 = xs.tile([P, IN_T, QBS], F16, tag="xq", name="xq_0")
            nc.sync.dma_start(
                xq0[:], xtq[:, 0:QBS].rearrange("(it p) t -> p it t", p=P))
            xv0 = xs.tile([P, IN_T, QBS], F16, tag="xv", name="xv_0")
            nc.sync.dma_start(
                xv0[:], xtv[:, 0:QBS].rearrange("(it p) t -> p it t", p=P))
            wv_sb = cpool.tile([P, IN_T, FL], F16)
            nc.sync.dma_start(wv_sb[:], wvt.rearrange("(it p) f -> p it f", p=P))
            wo_sb = cpool.tile([P, FT, d_model], F16)
            nc.sync.dma_start(wo_sb[:], wot.rearrange("(ft p) o -> p ft o", p=P))
            bqr_sb = cpool.tile([P, PAIRS], F32)
            nc.sync.dma_start(bqr_sb[:], bqr[:, :])
            bkr_sb = cpool.tile([P, PAIRS], F32)
            nc.sync.dma_start(bkr_sb[:], bkr[:, :])
            bv_sb = cpool.tile([1, FL], F16)
            nc.sync.dma_start(bv_sb[:], bvrow[:, :])
            ones1 = cpool.tile([1, P], F16)
            nc.gpsimd.memset(ones1[:], 1.0)
            # 4 diagonal 0/1 strips: strip j keeps (qi - kj_local - 128*j >= 0)
            mask4 = cpool.tile([P, KPB, QBS], F16)
            nc.gpsimd.memset(mask4[:], 1.0)
            for j in range(KPB):
                nc.gpsimd.affine_select(
                    out=mask4[:, j, :], in_=mask4[:, j, :],
                    compare_op=mybir.AluOpType.is_ge,
                    fill=0.0, base=-P * j,
                    pattern=[[1, QBS]], channel_multiplier=-1)

            # ---- persistent activations ----
            qT_sb = big.tile([P, PAIRS, seq], F16)   # [2-head f, pair, tok]
            kT_sb = big.tile([P, PAIRS, seq], F16)
            v_sb = big.tile([P, TT, HL, P], F16)  # [tok_in_tile, kj, h, d|1|pad]
            oT_sb = big.tile([P, FT, seq], F16)      # attention out, f-major

            nc.gpsimd.memset(v_sb[:], 0.0)
            nc.gpsimd.memset(v_sb[:, :, :, DK:DK + 1], 1.0)

            # warm the ACT exp table early (one-time ~2.7us load)
            es_warm = esp.tile([1, 8], F16, tag="warm")
            nc.scalar.activation(es_warm[:], ones1[0:1, 0:8], AF.Exp, scale=1.0)

            # One shared PSUM pool: tag "s" [128,2,512]x3 = 6 banks (scores,
            # projections, Wo) + tag "o" [65,512]x2 = 2 banks.
            pool_cm = tc.tile_pool(name="pmm", bufs=3, space="PSUM")
            pmm = pool_cm.__enter__()
            opool_cm = tc.tile_pool(name="po", bufs=2, space="PSUM")
            pop = opool_cm.__enter__()

            # qb-outer software pipeline: per 512-token chunk, project
            # K/Q/V for that chunk, run attention for all local heads on
            # that qi block, then the (partial) output projection for it.
            # Dense projection segments interleave with the ACT-paced
            # attention segments, keeping the PE HAM-warm throughout; the
            # next chunk's projections are emitted right after a chunk's
            # attention so they can fill attention-tail PE stalls.
            def emit_proj_dma(ch):
                    tsl = slice(ch * QBS, (ch + 1) * QBS)
                    xk_t = xs.tile([P, IN_T, QBS], F16, tag="xk",
                                   name=f"xk_{ch}")
                    nc.sync.dma_start(
                        xk_t[:], xtk[:, tsl].rearrange("(it p) t -> p it t", p=P))
                    xq_t = xs.tile([P, IN_T, QBS], F16, tag="xq",
                                   name=f"xq_{ch}")
                    nc.sync.dma_start(
                        xq_t[:], xtq[:, tsl].rearrange("(it p) t -> p it t", p=P))
                    xv_t = xs.tile([P, IN_T, QBS], F16, tag="xv",
                                   name=f"xv_{ch}")
                    nc.sync.dma_start(
                        xv_t[:], xtv[:, tsl].rearrange("(it p) t -> p it t", p=P))
                    return xk_t, xq_t, xv_t

            def proj_pieces(ch, tiles=None):
                    tsl = slice(ch * QBS, (ch + 1) * QBS)
                    xk_t, xq_t, xv_t = tiles if tiles else emit_proj_dma(ch)
                    pieces = []
                    def qk_piece(pair):
                        def go():
                            _emit_qk_pair(ch, tsl, xk_t, xq_t, pair)
                        return go
                    def v_piece(tl):
                        def go():
                            _emit_v_pair(ch, tsl, xv_t, tl)
                        return go
                    for pair in range(PAIRS):
                        pieces.append(qk_piece(pair))
                    for tl in range(0, KPB, 2):
                        pieces.append(v_piece(tl))
                    return pieces

            def emit_proj(ch, tiles=None):
                    for piece in proj_pieces(ch, tiles):
                        piece()

            def _emit_qk_pair(ch, tsl, xk_t, xq_t, pair):
                        fsl = slice(pair * P, (pair + 1) * P)
                        qk_ps = pmm.tile([P, 2, QBS], F32, tag="s",
                                         name=f"qk_{ch}_{pair}")
                        for it in range(IN_T):
                            nc.tensor.matmul(qk_ps[:, 0, :], wk_sb[:, it, fsl],
                                             xk_t[:, it, :],
                                             start=(it == 0), stop=(it == IN_T - 1))
                        for it in range(IN_T):
                            nc.tensor.matmul(qk_ps[:, 1, :], wq_sb[:, it, fsl],
                                             xq_t[:, it, :],
                                             start=(it == 0), stop=(it == IN_T - 1))
                        nc.vector.tensor_scalar_add(kT_sb[:, pair, tsl],
                                                    qk_ps[:, 0, :],
                                                    bkr_sb[:, pair:pair + 1])
                        nc.vector.tensor_scalar_add(qT_sb[:, pair, tsl],
                                                    qk_ps[:, 1, :],
                                                    bqr_sb[:, pair:pair + 1])

            def _emit_v_pair(ch, tsl, xv_t, tl):
                        v_ps = pmm.tile([P, 2, QBS], F32, tag="s",
                                        name=f"v_{ch}_{tl}")
                        for i in range(2):
                            tt = ch * KPB + tl + i
                            nc.tensor.matmul(v_ps[:, i, 0:FL], ones1[0:1, :],
                                             bv_sb[0:1, :], start=True, stop=False)
                            for it in range(IN_T):
                                nc.tensor.matmul(
                                    v_ps[:, i, 0:FL],
                                    xv_t[:, it, (tl + i) * P:(tl + i + 1) * P],
                                    wv_sb[:, it, :],
                                    start=False, stop=(it == IN_T - 1))
                        for i in range(2):
                            tt = ch * KPB + tl + i
                            nc.vector.tensor_copy(
                                v_sb[:, tt, :, 0:DK],
                                v_ps[:, i, 0:FL].rearrange("p (h d) -> p h d", h=HL))

            # ---- attention for qi block qb, all local heads ----
            def emit_attn(qb, fillers=()):
                fillers = list(fillers)
                qsl = slice(qb * QBS, (qb + 1) * QBS)
                KL = (qb + 1) * KPB if mode == "causal" else KJ
                for h in range(HL):
                    pair = h // 2
                    po = (h % 2) * DK
                    qT_h = qT_sb[po:po + DK, pair, qsl]
                    o_ps = pop.tile([P, QBS], F32, tag="o",
                                    name=f"o_{qb}_{h}")
                    for kj0 in range(0, KL, 2):
                        kjs = [kj0, kj0 + 1] if kj0 + 1 < KL else [kj0]
                        s_ps = pmm.tile([P, 2, QBS], F32, tag="s",
                                        name=f"s_{qb}_{h}_{kj0}")
                        for i, kj in enumerate(kjs):
                            nc.tensor.matmul(
                                s_ps[:, i, :],
                                kT_sb[po:po + DK, pair, kj * P:(kj + 1) * P],
                                qT_h, start=True, stop=True)
                        es = esp.tile([P, 2, QBS], F16, tag="es",
                                      name=f"es_{qb}_{h}_{kj0}")
                        n = len(kjs)
                        nc.scalar.activation(es[:, :n, :], s_ps[:, :n, :],
                                             AF.Exp, scale=0.125)
                        for i, kj in enumerate(kjs):
                            if mode == "causal" and kj // KPB == qb:
                                nc.vector.tensor_mul(es[:, i, :], es[:, i, :],
                                                     mask4[:, kj % KPB, :])
                            elif mode == "mask":
                                m_t = esp.tile([P, QBS], F16, tag="mt")
                                nc.sync.dma_start(
                                    m_t[:], maskt[kj * P:(kj + 1) * P, qsl])
                                nc.vector.tensor_mul(es[:, i, :], es[:, i, :],
                                                     m_t[:])
                        for i, kj in enumerate(kjs):
                            nc.tensor.matmul(o_ps[:], v_sb[:, kj, h, :],
                                             es[:, i, :],
                                             start=(kj == 0), stop=(kj == KL - 1))
                    # normalise by the softmax sum (row DK), f-major
                    srow = epi.tile([1, QBS], F32, tag="srow")
                    nc.vector.tensor_copy(srow[:], o_ps[DK:DK + 1, :])
                    recip_row = epi.tile([1, QBS], F32, tag="recip_row")
                    nc.vector.reciprocal_approx_fast(recip_row[:], srow[:])
                    recipb = epi.tile([DK, QBS], F32, tag="recipb")
                    nc.gpsimd.partition_broadcast(recipb[:], recip_row[0:1, :])
                    nc.vector.tensor_mul(oT_sb[po:po + DK, pair, qsl],
                                         o_ps[0:DK, :], recipb[:])
                    if fillers:
                        fillers.pop(0)()
                for f in fillers:
                    f()

            # ---- output projection for one token chunk ----
            def wo_pieces(qb):
                def tt_piece(tl):
                    def go():
                        _emit_wo_tt(qb, tl)
                    return go
                return [tt_piece(tl) for tl in range(KPB)]

            def emit_wo(qb):
                for piece in wo_pieces(qb):
                    piece()

            def _emit_wo_tt(qb, tl):
                    tt = qb * KPB + tl
                    w_ps = pmm.tile([P, 2, QBS], F32, tag="s", name=f"w_{tt}")
                    for ofc in range(OFC):
                        osl = slice(ofc * OFS, (ofc + 1) * OFS)
                        for ft in range(FT):
                            nc.tensor.matmul(w_ps[:, ofc, 0:OFS],
                                             oT_sb[:, ft, tt * P:(tt + 1) * P],
                                             wo_sb[:, ft, osl],
                                             start=(ft == 0), stop=(ft == FT - 1))
                    o_out = osb.tile([P, OFC, OFS], F32, tag="oo")
                    nc.vector.tensor_copy(o_out[:], w_ps[:, 0:OFC, 0:OFS])
                    nc.sync.dma_start(
                        out[tt * P:(tt + 1) * P, :],
                        o_out[:].rearrange("p c o -> p (c o)"))

            if mode == "causal":
                emit_proj(0, (xk0, xq0, xv0))
                if QB > 1:
                    emit_proj(1)
                for qb in range(QB):
                    if qb < 2:
                        fillers = []
                        if qb + 2 < QB:
                            fillers += proj_pieces(qb + 2)
                        if qb >= 1:
                            fillers += wo_pieces(qb - 1)
                        emit_attn(qb, fillers)
                    else:
                        emit_attn(qb)
                        if qb + 2 < QB:
                            emit_proj(qb + 2)
                        emit_wo(qb - 1)
                emit_wo(QB - 1)
            else:
                for ch in range(QB):
                    emit_proj(ch, (xk0, xq0, xv0) if ch == 0 else None)
                for qb in range(QB):
                    emit_attn(qb)
                    emit_wo(qb)

            opool_cm.__exit__(None, None, None)
            pool_cm.__exit__(None, None, None)

    nc.compile()
    return nc


_PROGRAMS = {}


def _get_program(mode, seq=SEQ, d_model=D_MODEL, num_heads=NUM_HEADS):
    key = (mode, seq, d_model, num_heads)
    if key not in _PROGRAMS:
        _PROGRAMS[key] = build_program(seq, d_model, num_heads, mode)
    return _PROGRAMS[key]


def _detect_mode(mask, seq):
    m = np.asarray(mask)
    if (m != 0).all():
        return "dense"
    tril = np.tril(np.ones((seq, seq), np.int8))
    if np.array_equal((m != 0).astype(np.int8), tril):
        return "causal"
    return "mask"


def prep_inputs(Q, K, V, mask, Wq, bq, Wk, bk, Wv, bv, Wo, bo,
                num_heads=NUM_HEADS, mode=None):
    batch, seq, d_model = Q.shape
    HL = num_heads // 2
    FL = HL * (d_model // num_heads)
    PAIRS = HL // 2
    if mode is None:
        mode = _detect_mode(mask, seq)
    maskt = None
    if mode == "mask":
        maskt = np.ascontiguousarray(
            (np.asarray(mask) != 0).astype(np.float16).T)
    in_maps = []
    for b in range(batch):
        xtq = Q[b].T.astype(np.float16)
        xtk = K[b].T.astype(np.float16)
        xtv = V[b].T.astype(np.float16)
        for half in range(2):
            fsl = slice(half * FL, (half + 1) * FL)
            im = {
                "xtq": xtq, "xtk": xtk, "xtv": xtv,
                "wqt": np.ascontiguousarray(Wq[fsl, :].T).astype(np.float16),
                "wkt": np.ascontiguousarray(Wk[fsl, :].T).astype(np.float16),
                "wvt": np.ascontiguousarray(Wv[fsl, :].T).astype(np.float16),
                "bqr": np.ascontiguousarray(
                    bq[fsl].reshape(PAIRS, P).T).astype(np.float32),
                "bkr": np.ascontiguousarray(
                    bk[fsl].reshape(PAIRS, P).T).astype(np.float32),
                "bvrow": bv[fsl].reshape(1, FL).astype(np.float16),
                "wot": np.ascontiguousarray(Wo[:, fsl].T).astype(np.float16),
            }
            if maskt is not None:
                im["maskt"] = maskt
            in_maps.append(im)
    return in_maps, mode


def _install_trace_hooks():
    """Provide antenv.axon_hooks (missing in this image) so that
    run_bass_kernel_spmd(trace=True) can capture NTFF profiles via the
    axon PJRT .so.  Bench-only; the graded path never enables tracing."""
    import contextlib
    import ctypes
    import types
    try:
        from antenv import axon_hooks  # noqa: F401
        return
    except ImportError:
        pass
    lib = ctypes.CDLL("/opt/axon/libaxon_pjrt.so")
    if not hasattr(lib, "axon_start_nrt_profile"):
        return
    lib.axon_start_nrt_profile.argtypes = [ctypes.POINTER(ctypes.c_int64),
                                           ctypes.c_size_t]
    lib.axon_start_nrt_profile.restype = ctypes.c_int64
    lib.axon_stop_nrt_profile.argtypes = [ctypes.c_char_p]
    lib.axon_stop_nrt_profile.restype = ctypes.c_int64

    @contextlib.contextmanager
    def _hook(output_dir, device_ids):
        import jax
        jax.devices()
        if device_ids:
            ids = (ctypes.c_int64 * len(device_ids))(*device_ids)
            rc = lib.axon_start_nrt_profile(ids, len(device_ids))
        else:
            rc = lib.axon_start_nrt_profile(None, 0)
        if rc != 0:
            raise RuntimeError(f"axon_start_nrt_profile rc={rc}")
        try:
            yield
        finally:
            n = lib.axon_stop_nrt_profile(str(output_dir).encode())
            print(f"profile: {n} file(s) written to {output_dir}", file=sys.stderr)

    mod = types.ModuleType("antenv.axon_hooks")
    mod.get_axon_ntff_profile_hook = lambda: _hook
    mod.set_axon_ntff_profile_hook = lambda h: None
    sys.modules["antenv.axon_hooks"] = mod
    import concourse.bass_utils as bu
    bu.upload_artifacts = lambda tmpdir: f"local:{tmpdir}"


def kernel(Q, K, V, mask, Wq, bq, Wk, bk, Wv, bv, Wo, bo):
    global LAST_EXEC_NS, LAST_RESULTS
    Q = np.asarray(Q); K = np.asarray(K); V = np.asarray(V)
    mask = np.asarray(mask)
    Wq = np.asarray(Wq, np.float32); bq = np.asarray(bq, np.float32)
    Wk = np.asarray(Wk, np.float32); bk = np.asarray(bk, np.float32)
    Wv = np.asarray(Wv, np.float32); bv = np.asarray(bv, np.float32)
    Wo = np.asarray(Wo, np.float32); bo = np.asarray(bo, np.float32)
    batch, seq, d_model = Q.shape

    in_maps, mode = prep_inputs(Q, K, V, mask, Wq, bq, Wk, bk, Wv, bv, Wo, bo)
    nc = _get_program(mode, seq, d_model, NUM_HEADS)

    trace = bool(os.environ.get("KBENCH_TRACE"))
    tmpdir = os.environ.get("KBENCH_TRACE_DIR") or None
    if trace:
        _install_trace_hooks()
    res = run_bass_kernel_spmd(nc, in_maps, list(range(N_CORES)), trace=trace,
                               tmpdir=tmpdir)
    LAST_EXEC_NS = res.exec_time_ns
    LAST_RESULTS = res
    out = np.empty((batch, seq, d_model), np.float32)
    for b in range(batch):
        out[b] = res.results[2 * b]["out"] + res.results[2 * b + 1]["out"] + bo
    return out
